# revision 6
# baseline (speedup 1.0000x reference)
import numpy as np

B = 128
FEAT = 64
LATENT = 512
OUT_F = 6144  # NUM_POINTS * 3
EPS = 1e-5
N_CORES = 8
SEGS_PER_CORE = 16
S_PAD = 8192
FMIN = np.float32(np.finfo(np.float32).min)

_CACHE = {}


def build_nc():
    from concourse import bass, bacc, tile

    mybir = bass.mybir
    f32 = mybir.dt.float32
    AF = mybir.ActivationFunctionType

    nc = bacc.Bacc("TRN2")
    xt_d = nc.declare_dram_parameter("xt", [128, 8, S_PAD], f32, isOutput=False)
    wp_d = nc.declare_dram_parameter("wp", [FEAT, LATENT], f32, isOutput=False)
    bp_d = nc.declare_dram_parameter("bp", [128, 4], f32, isOutput=False)
    w1_d = nc.declare_dram_parameter("w1p", [128, 1024], f32, isOutput=False)
    b1_d = nc.declare_dram_parameter("b1p", [128, 2], f32, isOutput=False)
    w2_d = nc.declare_dram_parameter("w2p", [128, 1024], f32, isOutput=False)
    b2_d = nc.declare_dram_parameter("b2p", [128, 4], f32, isOutput=False)
    w3_d = nc.declare_dram_parameter("w3p", [128, 48, 4, 128], f32, isOutput=False)
    b3_d = nc.declare_dram_parameter("b3p", [128, 48], f32, isOutput=False)
    out_d = nc.declare_dram_parameter("out", [128, 768], f32, isOutput=True)

    with tile.TileContext(nc) as tc:
        with (
            tc.tile_pool(name="wpool", bufs=1) as wpool,
            tc.tile_pool(name="fpool", bufs=2) as fpool,
            tc.tile_pool(name="spool", bufs=1) as spool,
            tc.tile_pool(name="w3pool", bufs=2) as w3pool,
            tc.tile_pool(name="ps_s", bufs=1, space=bass.MemorySpace.PSUM) as ps_s,
            tc.tile_pool(name="ps_b", bufs=1, space=bass.MemorySpace.PSUM) as ps_b,
            tc.tile_pool(name="ps_m", bufs=3, space=bass.MemorySpace.PSUM) as ps_m,
        ):
            wp = wpool.tile([FEAT, LATENT], f32)
            bp = wpool.tile([128, 4], f32)
            w1 = wpool.tile([128, 1024], f32)
            b1 = wpool.tile([128, 2], f32)
            w2 = wpool.tile([128, 1024], f32)
            b2 = wpool.tile([128, 4], f32)
            b3 = wpool.tile([128, 48], f32)
            for t, d in (
                (wp, wp_d), (bp, bp_d), (w1, w1_d), (b1, b1_d),
                (w2, w2_d), (b2, b2_d), (b3, b3_d),
            ):
                nc.sync.dma_start(t[:], d[:])

            ones_c = wpool.tile([FEAT, 1], f32)  # lhsT for column sums
            ones_r = wpool.tile([1, FEAT], f32)  # lhsT for partition broadcast
            eps_t = wpool.tile([1, 1], f32)
            nc.vector.memset(ones_c[:], 1.0)
            nc.vector.memset(ones_r[:], 1.0)
            nc.vector.memset(eps_t[:], EPS)

            val = spool.tile([128, 8], f32)
            z = spool.tile([FEAT, 16], f32)
            zc = spool.tile([FEAT, 16], f32)
            zsq = spool.tile([FEAT, 16], f32)
            znorm = spool.tile([FEAT, 16], f32)
            mu = spool.tile([1, 16], f32)
            std = spool.tile([1, 16], f32)
            rstd = spool.tile([1, 16], f32)
            lat = spool.tile([128, 64], f32)
            h1 = spool.tile([128, 32], f32)
            h2 = spool.tile([128, 64], f32)
            out_sb = spool.tile([128, 768], f32)

            # --- segment max pooling: 8 tiles of [128, S_PAD] ---
            for t in range(8):
                ft = fpool.tile([128, S_PAD], f32)
                nc.sync.dma_start(ft[:], xt_d[:, t, :])
                nc.vector.reduce_max(
                    val[:, t : t + 1], ft[:], axis=mybir.AxisListType.X
                )

            # assemble z [64 feats, 16 segs]
            nc.scalar.copy(z[:, 0:8], val[0:64, :])
            nc.sync.dma_start(z[:, 8:16], val[64:128, :])

            # --- LayerNorm (two-pass) ---
            sums = ps_s.tile([1, 16], f32)
            nc.tensor.matmul(sums[:], ones_c[:], z[:], start=True, stop=True)
            nc.scalar.mul(mu[:], sums[:], 1.0 / FEAT)
            mu_b = ps_b.tile([FEAT, 16], f32)
            nc.tensor.matmul(mu_b[:], ones_r[:], mu[:], start=True, stop=True)
            nc.vector.tensor_tensor(zc[:], z[:], mu_b[:], op=mybir.AluOpType.subtract)
            nc.scalar.activation(zsq[:], zc[:], AF.Square)
            varsum = ps_s.tile([1, 16], f32)
            nc.tensor.matmul(varsum[:], ones_c[:], zsq[:], start=True, stop=True)
            nc.scalar.activation(std[:], varsum[:], AF.Sqrt, bias=eps_t[:], scale=1.0 / FEAT)
            nc.vector.reciprocal(rstd[:], std[:])
            rstd_b = ps_b.tile([FEAT, 16], f32)
            nc.tensor.matmul(rstd_b[:], ones_r[:], rstd[:], start=True, stop=True)
            nc.vector.tensor_tensor(znorm[:], zc[:], rstd_b[:], op=mybir.AluOpType.mult)

            # --- proj (ln affine folded into wp/bp): lat[128m+p, s] ---
            for m in range(4):
                ps = ps_m.tile([128, 16], f32)
                nc.tensor.matmul(
                    ps[:], wp[:, 128 * m : 128 * (m + 1)], znorm[:],
                    start=True, stop=True,
                )
                nc.scalar.activation(
                    lat[:, 16 * m : 16 * (m + 1)], ps[:], AF.Identity,
                    bias=bp[:, m : m + 1],
                )

            # --- h1 = relu(latent @ w1 + b1), transposed ---
            for n in range(2):
                ps = ps_m.tile([128, 16], f32)
                for k in range(4):
                    nc.tensor.matmul(
                        ps[:],
                        w1[:, (k * 2 + n) * 128 : (k * 2 + n + 1) * 128],
                        lat[:, 16 * k : 16 * (k + 1)],
                        start=(k == 0), stop=(k == 3),
                    )
                nc.scalar.activation(
                    h1[:, 16 * n : 16 * (n + 1)], ps[:], AF.Relu,
                    bias=b1[:, n : n + 1],
                )

            # --- h2 = relu(h1 @ w2 + b2), transposed ---
            for n in range(4):
                ps = ps_m.tile([128, 16], f32)
                for k in range(2):
                    nc.tensor.matmul(
                        ps[:],
                        w2[:, (k * 4 + n) * 128 : (k * 4 + n + 1) * 128],
                        h1[:, 16 * k : 16 * (k + 1)],
                        start=(k == 0), stop=(k == 1),
                    )
                nc.scalar.activation(
                    h2[:, 16 * n : 16 * (n + 1)], ps[:], AF.Relu,
                    bias=b2[:, n : n + 1],
                )

            # --- out = h2 @ w3 + b3, streamed in 8 chunks of 6 n-blocks ---
            for grp in range(8):
                w3t = w3pool.tile([128, 6, 4, 128], f32)
                nc.sync.dma_start(w3t[:], w3_d[:, 6 * grp : 6 * (grp + 1), :, :])
                for j in range(6):
                    n = grp * 6 + j
                    ps = ps_m.tile([128, 16], f32)
                    for k in range(4):
                        nc.tensor.matmul(
                            ps[:], w3t[:, j, k, :], h2[:, 16 * k : 16 * (k + 1)],
                            start=(k == 0), stop=(k == 3),
                        )
                    nc.scalar.activation(
                        out_sb[:, 16 * n : 16 * (n + 1)], ps[:], AF.Identity,
                        bias=b3[:, n : n + 1],
                    )

            nc.sync.dma_start(out_d[:], out_sb[:])

    nc.finalize()
    return nc


def pack_weights(ln_g, ln_b, proj_w, proj_b, w1, b1, w2, b2, w3, b3):
    c = np.ascontiguousarray
    wp = c((ln_g[:, None] * proj_w).astype(np.float32))  # [64, 512]
    bpv = (ln_b.astype(np.float64) @ proj_w.astype(np.float64)).astype(np.float32) + proj_b
    return {
        "wp": wp,
        "bp": c(bpv.reshape(4, 128).T),
        "w1p": c(w1.reshape(4, 128, 2, 128).transpose(1, 0, 2, 3).reshape(128, 1024)),
        "b1p": c(b1.reshape(2, 128).T),
        "w2p": c(w2.reshape(2, 128, 4, 128).transpose(1, 0, 2, 3).reshape(128, 1024)),
        "b2p": c(b2.reshape(4, 128).T),
        "w3p": c(w3.reshape(4, 128, 48, 128).transpose(1, 2, 0, 3).reshape(128, 48, 4, 128)),
        "b3p": c(b3.reshape(48, 128).T),
    }


def pack_feat_core(feat, bounds, c):
    xt = np.full((128, 8, S_PAD), FMIN, np.float32)
    for sl in range(SEGS_PER_CORE):
        seg = c * SEGS_PER_CORE + sl
        a, b = bounds[seg], bounds[seg + 1]
        blk = feat[a:b]
        L = b - a
        if L > S_PAD:
            blk = np.concatenate(
                [blk[: S_PAD - 1], blk[S_PAD - 1 :].max(0, keepdims=True)], 0
            )
            L = S_PAD
        g, t = divmod(sl, 8)
        if L > 0:
            xt[g * 64 : (g + 1) * 64, t, :L] = blk.T
    return xt


def kernel(**inputs):
    from concourse.bass_utils import run_bass_kernel_spmd

    feat = np.ascontiguousarray(np.asarray(inputs["feat"], dtype=np.float32))
    batch = np.asarray(inputs["batch"])
    wdict = pack_weights(
        *(np.asarray(inputs[k], dtype=np.float32) for k in
          ("ln_g", "ln_b", "proj_w", "proj_b", "w1", "b1", "w2", "b2", "w3", "b3"))
    )

    if "nc" not in _CACHE:
        _CACHE["nc"] = build_nc()
    nc = _CACHE["nc"]

    bounds = np.searchsorted(batch, np.arange(B + 1))
    in_maps = [
        {"xt": pack_feat_core(feat, bounds, c), **wdict} for c in range(N_CORES)
    ]
    res = run_bass_kernel_spmd(nc, in_maps, list(range(N_CORES)))

    out = np.empty((B, OUT_F), np.float32)
    for c in range(N_CORES):
        oc = res.results[c]["out"]  # [128, 768]
        out[c * 16 : (c + 1) * 16] = (
            oc.reshape(128, 48, 16).transpose(2, 1, 0).reshape(16, OUT_F)
        )
    return out.reshape(B, 2048, 3)
